# revision 1
# baseline (speedup 1.0000x reference)
"""BinarizedLinear TRN2 kernel: y = x @ sign(weight).T + bias.

Full shapes: x [8192, 4096] f32, weight [4096, 4096] f32, bias [4096] f32
-> y [8192, 4096] f32.

Sharding across 8 NeuronCores: tokens split 2 ways x out_features split 4
ways. Each core computes a [4096, 1024] output block. The transposed
weight shard (16 MB) stays SBUF-resident, binarized on-device via the ACT
Sign LUT into float32r; x streams in K-major strips cast to float32r by
SWDGE cast-DMAs; TensorE runs single-pass float32r matmuls (full
bf16-rate) accumulating in fp32 PSUM; bias is added on PSUM eviction.
Host does layout only (transpose/tile/slice); sign, matmul and bias run
on device.
"""
import sys

if "/opt/trn_rl_repo" not in sys.path:
    sys.path.insert(0, "/opt/trn_rl_repo")

import numpy as np
import concourse.bass as bass
import concourse.mybir as mybir
import concourse.tile as tile
from concourse.bass_utils import run_bass_kernel_spmd

TOKENS, IN_F, OUT_F = 8192, 4096, 4096
T_SHARDS, O_SHARDS = 2, 4
TOK_PER = TOKENS // T_SHARDS  # 4096 tokens per core
OUT_PER = OUT_F // O_SHARDS   # 1024 out features per core
P = 128
KT = IN_F // P                # 32 contraction tiles
TT = TOK_PER // P             # 32 token tiles
NH = OUT_PER // 512           # 2 psum-bank halves

F32 = mybir.dt.float32
F32R = mybir.dt.float32r


def split_excess_waits(nc, max_waits=1):
    """This walrus build encodes at most one semaphore wait per
    instruction; move excess waits onto preceding same-engine NoOps."""
    ctr = 0
    for fn in nc.m.functions:
        for bb in fn.blocks:
            insts = bb.instructions
            i = 0
            while i < len(insts):
                inst = insts[i]
                si = getattr(inst, "sync_info", None)
                ow = list(si.on_wait) if si else []
                if len(ow) > max_waits:
                    extra, keep = ow[:-max_waits], ow[-max_waits:]
                    si.on_wait = keep
                    inst.sync_info = si
                    k = 0
                    for j in range(0, len(extra), max_waits):
                        ctr += 1
                        nop = mybir.InstNoOp(
                            name=f"I-waitsplit-{ctr}", ins=[], outs=[]
                        )
                        nop.engine = inst.engine
                        nop.sync_info = mybir.SyncInfo(
                            on_wait=extra[j : j + max_waits], on_update=[]
                        )
                        insts.insert(i + k, nop)
                        k += 1
                    i += k
                i += 1
    return ctr


def build_nc():
    nc = bass.Bass()
    # xs: x shard pre-tiled on host to [TT, P(k_lo), KT*P(t-major)] so each
    # SBUF partition reads one contiguous 16 KB run per strip DMA.
    xs = nc.dram_tensor("xs", [TT, P, KT * P], F32, kind="ExternalInput")
    wT = nc.dram_tensor("wT", [IN_F, OUT_PER], F32, kind="ExternalInput")
    biasb = nc.dram_tensor("biasb", [P, OUT_PER], F32, kind="ExternalInput")
    y = nc.dram_tensor("y", [TOK_PER, OUT_PER], F32, kind="ExternalOutput")

    wT_r = wT.rearrange("(ko p) o -> p ko o", p=P)

    with tile.TileContext(nc) as tc:
        with (
            tc.tile_pool(name="wres", bufs=1) as wres_pool,
            tc.tile_pool(name="xr", bufs=4) as xr_pool,
            tc.tile_pool(name="outp", bufs=2) as out_pool,
            tc.tile_pool(name="psum", bufs=8, space="PSUM") as psum_pool,
        ):
            def x_quarter(xr, t, j):
                # SWDGE cast-DMA: f32 DRAM -> float32r SBUF (rounds).
                # Quarter-strip sub-DMAs; x and w share the SWDGE FIFO,
                # so emission order paces the HBM bandwidth split.
                q = KT // 4
                nc.gpsimd.dma_start(
                    xr[:, j * q : (j + 1) * q, :].rearrange("p k t -> p (k t)"),
                    xs[t, :, j * q * P : (j + 1) * q * P],
                )

            def load_x_strip(t):
                xr = xr_pool.tile([P, KT, P], F32R, tag="xr")
                for j in range(4):
                    x_quarter(xr, t, j)
                return xr

            # First x strip: quarter 0 ahead of the weight stream so the
            # first matmul group can start immediately; remaining quarters
            # interleave with the first weight tiles. Strips 1-3 are
            # injected into the weight stream so early matmul groups ramp
            # up without starving the 16 MB weight load.
            x0 = xr_pool.tile([P, KT, P], F32R, tag="xr")
            x_strips = {0: x0}
            x_quarter(x0, 0, 0)

            w_tiles = []
            quarter_at = {0: 1, 1: 2, 2: 3}
            prefetch_at = {4: 1, 9: 2, 15: 3}
            for k in range(KT):
                wt = wres_pool.tile([P, OUT_PER], F32R, tag=f"w{k}")
                nc.gpsimd.dma_start(wt[:], wT_r[:, k, :])
                nc.scalar.sign(wt[:], wt[:])
                w_tiles.append(wt)
                if k in quarter_at:
                    x_quarter(x0, 0, quarter_at[k])
                if k in prefetch_at:
                    t = prefetch_at[k]
                    x_strips[t] = load_x_strip(t)

            bias_sb = wres_pool.tile([P, OUT_PER], F32, tag="bias")
            nc.sync.dma_start(bias_sb[:], biasb[:])

            for t in range(TT):
                xr = x_strips.pop(t)
                if t + 4 < TT:
                    x_strips[t + 4] = load_x_strip(t + 4)

                for oh in range(NH):
                    ps = psum_pool.tile([P, 512], F32, tag="ps")
                    for k in range(KT):
                        nc.tensor.matmul(
                            ps[:],
                            xr[:, k, :],
                            w_tiles[k][:, oh * 512 : (oh + 1) * 512],
                            start=(k == 0),
                            stop=(k == KT - 1),
                        )
                    out_sb = out_pool.tile([P, 512], F32, tag="out")
                    nc.vector.tensor_add(
                        out_sb[:],
                        ps[:],
                        bias_sb[:, oh * 512 : (oh + 1) * 512],
                    )
                    nc.sync.dma_start(
                        y[t * P : (t + 1) * P, oh * 512 : (oh + 1) * 512],
                        out_sb[:],
                    )

    split_excess_waits(nc)
    return nc


_NC = None


def _get_nc():
    global _NC
    if _NC is None:
        _NC = build_nc()
    return _NC


def make_in_maps(x, weight, bias):
    x = np.asarray(x, dtype=np.float32)
    weight = np.asarray(weight, dtype=np.float32)
    bias = np.asarray(bias, dtype=np.float32)
    wT = np.ascontiguousarray(weight.T)  # [IN_F, OUT_F]
    in_maps = []
    for c in range(8):
        th, oq = divmod(c, O_SHARDS)
        xsh = x[th * TOK_PER : (th + 1) * TOK_PER]  # [TOK_PER, IN_F]
        # [TT, P_t, KT, P_k] -> [TT, P_k, KT, P_t]: partition dim = k_lo,
        # contiguous 16 KB per partition per strip
        xt = np.ascontiguousarray(
            xsh.reshape(TT, P, KT, P).transpose(0, 3, 2, 1)
        ).reshape(TT, P, KT * P)
        in_maps.append(
            {
                "xs": xt,
                "wT": np.ascontiguousarray(
                    wT[:, oq * OUT_PER : (oq + 1) * OUT_PER]
                ),
                "biasb": np.ascontiguousarray(
                    np.broadcast_to(
                        bias[oq * OUT_PER : (oq + 1) * OUT_PER], (P, OUT_PER)
                    )
                ),
            }
        )
    return in_maps


def assemble(results):
    out = np.empty((TOKENS, OUT_F), np.float32)
    for c in range(8):
        th, oq = divmod(c, O_SHARDS)
        out[
            th * TOK_PER : (th + 1) * TOK_PER,
            oq * OUT_PER : (oq + 1) * OUT_PER,
        ] = results[c]["y"]
    return out


def kernel(x, weight, bias):
    in_maps = make_in_maps(x, weight, bias)
    res = run_bass_kernel_spmd(_get_nc(), in_maps, core_ids=list(range(8)))
    return assemble(res.results)



# revision 3
# speedup vs baseline: 1.0517x; 1.0517x over previous
"""BinarizedLinear TRN2 kernel: y = x @ sign(weight).T + bias.

Full shapes: x [8192, 4096] f32, weight [4096, 4096] f32, bias [4096] f32
-> y [8192, 4096] f32.

Sharding across 8 NeuronCores: tokens split 2 ways x out_features split 4
ways. Each core computes a [4096, 1024] output block. Both matmul
operands are bf16: the weight shard streams through a small f32 staging
pool and is binarized by the ACT Sign LUT directly into resident bf16
tiles (sign values +-1 are exact in bf16); x streams in K-major strips
cast f32->bf16 by SWDGE cast-DMAs. bf16 stationary tiles get
fast-weight-load LDWEIGHTS, so the MM cadence approaches the N=512
streaming floor. TensorE accumulates in fp32 PSUM; bias is added on PSUM
eviction. Host does layout only (transpose/tile/slice); sign, matmul and
bias run on device.
"""
import sys

if "/opt/trn_rl_repo" not in sys.path:
    sys.path.insert(0, "/opt/trn_rl_repo")

import numpy as np
import concourse.bass as bass
import concourse.mybir as mybir
import concourse.tile as tile
from concourse.bass_utils import run_bass_kernel_spmd

TOKENS, IN_F, OUT_F = 8192, 4096, 4096
T_SHARDS, O_SHARDS = 2, 4
TOK_PER = TOKENS // T_SHARDS  # 4096 tokens per core
OUT_PER = OUT_F // O_SHARDS   # 1024 out features per core
P = 128
KT = IN_F // P                # 32 contraction tiles
TT = TOK_PER // P             # 32 token tiles
NH = OUT_PER // 512           # 2 psum-bank halves
XBUFS = 8                     # x strip prefetch depth

F32 = mybir.dt.float32
BF16 = mybir.dt.bfloat16


def split_excess_waits(nc, max_waits=1):
    """This walrus build encodes at most one semaphore wait per
    instruction; move excess waits onto preceding same-engine NoOps."""
    ctr = 0
    for fn in nc.m.functions:
        for bb in fn.blocks:
            insts = bb.instructions
            i = 0
            while i < len(insts):
                inst = insts[i]
                si = getattr(inst, "sync_info", None)
                ow = list(si.on_wait) if si else []
                if len(ow) > max_waits:
                    extra, keep = ow[:-max_waits], ow[-max_waits:]
                    si.on_wait = keep
                    inst.sync_info = si
                    k = 0
                    for j in range(0, len(extra), max_waits):
                        ctr += 1
                        nop = mybir.InstNoOp(
                            name=f"I-waitsplit-{ctr}", ins=[], outs=[]
                        )
                        nop.engine = inst.engine
                        nop.sync_info = mybir.SyncInfo(
                            on_wait=extra[j : j + max_waits], on_update=[]
                        )
                        insts.insert(i + k, nop)
                        k += 1
                    i += k
                i += 1
    return ctr


def build_nc():
    nc = bass.Bass()
    # xs: x shard pre-tiled on host to [TT, P(k_lo), KT*P(t-major)] so each
    # SBUF partition reads one contiguous 16 KB run per strip DMA.
    xs = nc.dram_tensor("xs", [TT, P, KT * P], F32, kind="ExternalInput")
    wT = nc.dram_tensor("wT", [IN_F, OUT_PER], F32, kind="ExternalInput")
    biasb = nc.dram_tensor("biasb", [P, OUT_PER], F32, kind="ExternalInput")
    y = nc.dram_tensor("y", [TOK_PER, OUT_PER], F32, kind="ExternalOutput")

    wT_r = wT.rearrange("(ko p) o -> p ko o", p=P)

    with tile.TileContext(nc) as tc:
        with (
            tc.tile_pool(name="wbin", bufs=1) as wbin_pool,
            tc.tile_pool(name="wstg", bufs=4) as wstg_pool,
            tc.tile_pool(name="xr", bufs=XBUFS) as xr_pool,
            tc.tile_pool(name="outp", bufs=4) as out_pool,
            tc.tile_pool(name="psum", bufs=8, space="PSUM") as psum_pool,
        ):
            Q = KT // 4  # 8 k-tiles per quarter-strip DMA

            def x_quarter(xr, t, j):
                # SWDGE cast-DMA: f32 DRAM -> bf16 SBUF (rounds). x and w
                # share the SWDGE FIFO, so emission order paces the HBM
                # bandwidth split.
                nc.gpsimd.dma_start(
                    xr[:, j * Q : (j + 1) * Q, :].rearrange("p k t -> p (k t)"),
                    xs[t, :, j * Q * P : (j + 1) * Q * P],
                )

            def new_strip():
                return xr_pool.tile([P, KT, P], BF16, tag="xr", name="xr")

            def load_x_strip(t):
                xr = new_strip()
                for j in range(4):
                    x_quarter(xr, t, j)
                return xr

            def load_w(k, halves=False):
                # stage f32 tile, binarize via ACT Sign into resident bf16
                stg = wstg_pool.tile([P, OUT_PER], F32, tag="wstg")
                wb = wbin_pool.tile([P, OUT_PER], BF16, tag=f"wb{k}")
                if halves:
                    for h in range(2):
                        sl = slice(h * 512, (h + 1) * 512)
                        nc.gpsimd.dma_start(stg[:, sl], wT_r[:, k, sl])
                        nc.scalar.sign(wb[:, sl], stg[:, sl])
                else:
                    nc.gpsimd.dma_start(stg[:], wT_r[:, k, :])
                    for h in range(2):
                        sl = slice(h * 512, (h + 1) * 512)
                        nc.scalar.sign(wb[:, sl], stg[:, sl])
                return wb

            # bias via HWDGE on the sync queue: off the SWDGE FIFO, lands
            # in the first ~10us without displacing x/w bytes.
            bias_sb = wbin_pool.tile([P, OUT_PER], F32, tag="bias")
            nc.sync.dma_start(bias_sb[:], biasb[:])

            # Startup: first x k-slice (64 KB) and first w half-tile lead
            # the FIFO so MM(t0,oh0,k0) can issue ~1us after data flows;
            # then interleave ~one x quarter per w tile so 6 strips are
            # resident by the time the weight stream finishes.
            x0 = new_strip()
            nc.gpsimd.dma_start(
                x0[:, 0:1, :].rearrange("p k t -> p (k t)"), xs[0, :, 0:P]
            )
            w_tiles = [load_w(0, halves=True)]
            nc.gpsimd.dma_start(
                x0[:, 1:Q, :].rearrange("p k t -> p (k t)"), xs[0, :, P : Q * P]
            )
            x_strips = {0: x0}
            quarter_at = {1: 1, 2: 2, 3: 3}  # remaining x0 quarters
            for k in range(1, KT):
                w_tiles.append(load_w(k))
                if k in quarter_at:
                    x_quarter(x0, 0, quarter_at[k])
                elif 4 <= k < 24:
                    # strips 1-5: one quarter per w tile
                    t, j = divmod(k - 4, 4)
                    if j == 0:
                        x_strips[1 + t] = new_strip()
                    x_quarter(x_strips[1 + t], 1 + t, j)

            # strips 6,7 queue behind the weight stream
            for t in (6, 7):
                x_strips[t] = load_x_strip(t)

            for t in range(TT):
                xr = x_strips.pop(t)
                if t + XBUFS < TT:
                    x_strips[t + XBUFS] = load_x_strip(t + XBUFS)

                for oh in range(NH):
                    ps = psum_pool.tile([P, 512], F32, tag="ps")
                    for k in range(KT):
                        nc.tensor.matmul(
                            ps[:],
                            xr[:, k, :],
                            w_tiles[k][:, oh * 512 : (oh + 1) * 512],
                            start=(k == 0),
                            stop=(k == KT - 1),
                        )
                    out_sb = out_pool.tile([P, 512], F32, tag="out")
                    nc.vector.tensor_add(
                        out_sb[:],
                        ps[:],
                        bias_sb[:, oh * 512 : (oh + 1) * 512],
                    )
                    nc.sync.dma_start(
                        y[t * P : (t + 1) * P, oh * 512 : (oh + 1) * 512],
                        out_sb[:],
                    )

    split_excess_waits(nc)
    return nc


_NC = None


def _get_nc():
    global _NC
    if _NC is None:
        _NC = build_nc()
    return _NC


def make_in_maps(x, weight, bias):
    x = np.asarray(x, dtype=np.float32)
    weight = np.asarray(weight, dtype=np.float32)
    bias = np.asarray(bias, dtype=np.float32)
    wT = np.ascontiguousarray(weight.T)  # [IN_F, OUT_F]
    in_maps = []
    for c in range(8):
        th, oq = divmod(c, O_SHARDS)
        xsh = x[th * TOK_PER : (th + 1) * TOK_PER]  # [TOK_PER, IN_F]
        # [TT, P_t, KT, P_k] -> [TT, P_k, KT, P_t]: partition dim = k_lo,
        # contiguous 16 KB per partition per strip
        xt = np.ascontiguousarray(
            xsh.reshape(TT, P, KT, P).transpose(0, 3, 2, 1)
        ).reshape(TT, P, KT * P)
        in_maps.append(
            {
                "xs": xt,
                "wT": np.ascontiguousarray(
                    wT[:, oq * OUT_PER : (oq + 1) * OUT_PER]
                ),
                "biasb": np.ascontiguousarray(
                    np.broadcast_to(
                        bias[oq * OUT_PER : (oq + 1) * OUT_PER], (P, OUT_PER)
                    )
                ),
            }
        )
    return in_maps


def assemble(results):
    out = np.empty((TOKENS, OUT_F), np.float32)
    for c in range(8):
        th, oq = divmod(c, O_SHARDS)
        out[
            th * TOK_PER : (th + 1) * TOK_PER,
            oq * OUT_PER : (oq + 1) * OUT_PER,
        ] = results[c]["y"]
    return out


def kernel(x, weight, bias):
    in_maps = make_in_maps(x, weight, bias)
    res = run_bass_kernel_spmd(_get_nc(), in_maps, core_ids=list(range(8)))
    return assemble(res.results)
